# revision 3
# baseline (speedup 1.0000x reference)
"""Trainium2 Bass kernel for the non-local (self-attention over spatial
positions) block.

Per batch b (8 batches -> one per NeuronCore):
    xf    = x[b]                       [C=128, N=4096]
    theta = w_theta @ xf               [64, N]
    phi   = w_phi   @ xf               [64, N]
    g     = w_g     @ xf               [64, N]
    attn  = softmax(theta^T phi)       [N, N]   (softmax over keys m)
    y     = g @ attn^T                 [64, N]
    out   = w_last @ y + xf            [128, N]

Design (per core):
 - scoresT orientation: scoresT[m, q] = sum_k phi[k,m] theta[k,n] computed
   with phi m-tiles as the stationary operand, so exp(scoresT) feeds the
   second matmul directly as the moving operand (no transposes).
 - No max-subtraction: logits ~ N(0,64) can't overflow f32 exp.
 - Row sums come for free from a ones column appended to gT (the stationary
   operand of the y matmul); normalization happens on the [64+1, q] result.
 - Big matmul in float32r (1 cycle/row), probs/y in bf16, everything else
   f32. exp on ACT, copies on DVE, normalizer broadcast on GPSIMD.
"""

import sys

import numpy as np

for _p in ("/opt/trn_rl_repo",):
    if _p not in sys.path:
        sys.path.insert(0, _p)

import concourse.bass as bass
from concourse import bacc
import concourse.mybir as mybir
import concourse.tile as tile
from concourse.bass_utils import run_bass_kernel_spmd

F32 = mybir.dt.float32
F32R = mybir.dt.float32r
BF16 = mybir.dt.bfloat16

P = 128     # channels C / partition dim
CB = 64     # bottleneck channels
NQ = 4096   # spatial positions (64*64)
QT = 1024   # query-tile (quarter) size
NQQ = NQ // QT
MT = 32     # m (key) tiles of 128

_NC_CACHE = {}


def _build():
    nc = bacc.Bacc()
    x_in = nc.declare_dram_parameter("xb", [P, NQ], F32, isOutput=False)
    wqk_in = nc.declare_dram_parameter("wqk", [P, P], F32, isOutput=False)
    wg_in = nc.declare_dram_parameter("wgT", [P, CB], F32, isOutput=False)
    wl_in = nc.declare_dram_parameter("wl", [CB, P], F32, isOutput=False)
    out_d = nc.declare_dram_parameter("out", [P, NQ], F32, isOutput=True)

    with tile.TileContext(nc) as tc:
        with (
            tc.tile_pool(name="const", bufs=1) as const,
            tc.tile_pool(name="big", bufs=1) as big,
            tc.tile_pool(name="work", bufs=2) as work,
            tc.tile_pool(name="probs", bufs=3) as probs,
            tc.tile_pool(name="pps", bufs=2, space="PSUM") as pps,
            tc.tile_pool(name="spool", bufs=2, space="PSUM") as spool,
            tc.tile_pool(name="ypool", bufs=1, space="PSUM") as ypool,
        ):
            # ---- loads ----
            xb = big.tile([P, NQ], F32)
            for j in range(8):
                nc.sync.dma_start(
                    out=xb[:, j * 512:(j + 1) * 512],
                    in_=x_in[:, j * 512:(j + 1) * 512],
                )
            wqk = const.tile([P, P], F32)
            wg = const.tile([P, CB], F32)
            wl = const.tile([CB, P], F32)
            nc.sync.dma_start(out=wqk, in_=wqk_in[:, :])
            nc.sync.dma_start(out=wg, in_=wg_in[:, :])
            nc.sync.dma_start(out=wl, in_=wl_in[:, :])
            wlr = const.tile([CB, P], F32R)
            nc.vector.tensor_copy(wlr, wl)

            # ---- projections: theta/phi (f32r tiles), gT+ones (bf16) ----
            theta = big.tile([CB, NQ], F32R)
            phi = big.tile([CB, NQ], F32R)
            for j in range(8):
                ps = pps.tile([P, 512], F32, tag="pps")
                nc.tensor.matmul(
                    ps, wqk, xb[:, j * 512:(j + 1) * 512], start=True, stop=True
                )
                nc.vector.tensor_copy(theta[:, j * 512:(j + 1) * 512], ps[0:CB, :])
                nc.vector.tensor_copy(phi[:, j * 512:(j + 1) * 512], ps[CB:P, :])

            gt = big.tile([P, MT * (CB + 1)], BF16)
            nc.vector.memset(gt, 1.0)
            for mi in range(MT):
                gp = pps.tile([P, 512], F32, tag="pps")
                nc.tensor.matmul(
                    gp[:, 0:CB], xb[:, mi * 128:(mi + 1) * 128], wg,
                    start=True, stop=True,
                )
                nc.vector.tensor_copy(
                    gt[:, mi * (CB + 1):mi * (CB + 1) + CB], gp[:, 0:CB]
                )

            # ---- main attention loop over query quarters ----
            for qq in range(NQQ):
                q0 = qq * QT
                yps = ypool.tile([CB + 1, QT], F32, tag="y")
                for mi in range(MT):
                    sp = spool.tile([P, QT], F32, tag="s")
                    for j in range(2):
                        nc.tensor.matmul(
                            sp[:, j * 512:(j + 1) * 512],
                            phi[:, mi * 128:(mi + 1) * 128],
                            theta[:, q0 + j * 512:q0 + (j + 1) * 512],
                            start=True, stop=True,
                        )
                    pb = probs.tile([P, QT], BF16, tag="pb")
                    nc.scalar.activation(pb, sp, mybir.ActivationFunctionType.Exp)
                    for j in range(2):
                        nc.tensor.matmul(
                            yps[:, j * 512:(j + 1) * 512],
                            gt[:, mi * (CB + 1):(mi + 1) * (CB + 1)],
                            pb[:, j * 512:(j + 1) * 512],
                            start=(mi == 0), stop=(mi == MT - 1),
                        )

                # ---- normalize + final projection + residual ----
                rinv = work.tile([1, QT], F32, tag="rinv")
                nc.vector.reciprocal(rinv, yps[CB:CB + 1, :])
                rb = work.tile([CB, QT], F32, tag="rb")
                nc.gpsimd.partition_broadcast(rb, rinv)
                yn = work.tile([CB, QT], F32R, tag="yn")
                nc.vector.tensor_mul(yn, yps[0:CB, :], rb)

                op = spool.tile([P, QT], F32, tag="s")
                for j in range(2):
                    nc.tensor.matmul(
                        op[:, j * 512:(j + 1) * 512], wlr,
                        yn[:, j * 512:(j + 1) * 512],
                        start=True, stop=True,
                    )
                ob = work.tile([P, QT], F32, tag="ob")
                nc.vector.tensor_add(ob, op, xb[:, q0:q0 + QT])
                nc.sync.dma_start(out=out_d[:, q0:q0 + QT], in_=ob)

    nc.finalize()
    return nc


def kernel(x, w_theta, w_phi, w_g, w_last):
    B, C, H, W = x.shape
    N = H * W
    xf = np.ascontiguousarray(x.reshape(B, C, N), dtype=np.float32)
    wqk = np.ascontiguousarray(
        np.concatenate([w_theta.T, w_phi.T], axis=1), dtype=np.float32
    )
    wgT = np.ascontiguousarray(w_g.T, dtype=np.float32)
    wl = np.ascontiguousarray(w_last.T, dtype=np.float32)

    if "nc" not in _NC_CACHE:
        _NC_CACHE["nc"] = _build()
    nc = _NC_CACHE["nc"]

    in_maps = [
        {"xb": xf[b], "wqk": wqk, "wgT": wgT, "wl": wl} for b in range(B)
    ]
    r = run_bass_kernel_spmd(nc, in_maps, list(range(B)))
    out = np.stack([r.results[b]["out"] for b in range(B)], axis=0)
    return out.reshape(B, C, H, W).astype(np.float32)
